# revision 15
# baseline (speedup 1.0000x reference)
"""BlockDecay (RetNet-style chunkwise linear attention with per-feature decay)
Trainium2 Bass kernel, batch-parallel over 8 NeuronCores.

Math (per batch): out[t] = sum_r q[t,r] * S_t[r,:],
  S_t[r,d] = sum_{s<=t} gamma_r^{t-s} k[s,r] h[s,d]
computed chunkwise with C=128 using the standard factorization
  A[i,j] = (q gamma^i) . (k gamma^-j),  intra = (A*mask) @ h,
  inter  = (q gamma^i) @ S,   S' = gamma^C S + K',
  K'[r,d] = sum_j gamma_r^{C-j} k[j,r] h[j,d]   (S carries a folded
  gamma^1 so inter needs no extra scale)

v17: all operands bf16 (PE 1 cyc/row + FWL weight loads; PSUM
accumulates fp32) with the PSUM->SBUF crossings batched and the PE
queue software-pipelined:
  - A blocks for 8 chunks land in the low half of one [128,2048] PSUM
    tile; ONE DVE mask-multiply per group against a x8-replicated tri.
  - K' blocks ride the tile's high half (stt reads PSUM directly), so
    no dedicated K' pool and the serial state chain starts as soon as
    each chunk's data lands (128-col head DMA pieces).
  - Output-block matmuls are emitted one mask-group late (pend) so the
    PE never idles behind the mask; OT copies batch x4 on ACT; output
    DMA pieces ride scalar's own HWDGE queue (sharing an input queue
    was measured to halve that queue's bandwidth).
Group sizes taper (8/8/8/6/2) to keep the tail chain short.

Host pre-scales/transposes all operands; device layout:
  qsT [R, W] bf16 = (q * gamma^(i%C)).T
  ksT [R, W] bf16 = (k * gamma^-(j%C)).T
  k2n [128, W] bf16  block-local [j, (blk, r)] = k*gamma^(C - j%C)
  hn  [128, W] bf16  block-local [j, (blk, d)]
  tri [128, 128] f32 causal mask transposed (tri[j,i] = i>=j)
  g128 [128, 1] f32 = gamma^C
Output otT [D, W] bf16 (transposed), host transposes + upcasts.
Measured rel err (absmax norm) ~4e-3 vs the 2e-2 gate.
"""
import os
import sys
import numpy as np

for _p in ("/root/.axon_site", "/root/.axon_site/_ro/trn_rl_repo",
           "/root/.axon_site/_ro/pypackages"):
    if _p not in sys.path and os.path.isdir(_p):
        sys.path.append(_p)

B, W, R, D = 8, 4096, 128, 128
C = 128
NBLK = W // C

G_MSK = [(0, 8), (8, 16), (16, 24), (24, 30), (30, 32)]
G_OUT = [(0, 4), (4, 8), (8, 12), (12, 16), (16, 20), (20, 24), (24, 28),
         (28, 30), (30, 31), (31, 32)]
# output DMA piece [lo,hi) chunks, issued after the OT copy ending at `cb`
_OUT_PIECES = {8: (0, 8), 16: (8, 16), 24: (16, 24), 28: (24, 28),
               30: (28, 30), 31: (30, 31), 32: (31, 32)}
# input DMA piece boundaries (cols): small head pieces start compute
# ~2.5us earlier; 512-col steady pieces keep PE gaps under the ~3.4us
# HAM re-throttle window
IN_BOUNDS = [0, 128, 256, 512, 1024, 1536, 2048, 2560, 3072, 3584, 4096]

_PROG = {}


def _patched_tc(nc):
    """TileContext with a cheap exit: per-sem single-wait drains on sync
    (this walrus accepts one sync-wait per instruction, and a blocking
    drain on an early-finishing engine stalls SWDGE descriptor handling),
    one barrier, then sem clears for idempotent re-execution.  The final
    join is walrus's own BSP model-end sync."""
    import concourse.tile as tile
    import concourse.tile_sem_assignment as tsa
    from concourse.tile import ScopedClock

    class PatchedTileContext(tile.TileContext):
        def _drain_and_barrier(self, tick_clock, wait_clock):
            gc = tick_clock.global_clock
            n = tsa.N_PROCS
            nc = self.nc
            for p in range(n):
                ticks = gc[p]
                if ticks <= 0:
                    continue
                d = nc.sync.drain()
                wait_clock.add_sem_waits(
                    d.ins,
                    ScopedClock({None: tsa.VectorClock(
                        [ticks if q == p else 0 for q in range(n)])}),
                )
            nc.all_engine_barrier()
            assert self.sems is not None
            popped = nc._tile_sem_poison_stack.pop()
            assert popped is self._sem_poison
            nc.clear_and_free_semaphores(list(self.sems.allocated().values()))

    return PatchedTileContext(nc)


def _split_multi_waits(nc, limit=1):
    """Hoist extra sync-waits onto injected same-engine NoOps (in-order
    engines make waiting earlier in the stream safe)."""
    import concourse.mybir as mybir
    n_new = 0
    for fn in nc.m.functions:
        for bb in fn.blocks:
            out = []
            changed = False
            for inst in bb.instructions:
                si = getattr(inst, "sync_info", None)
                waits = list(si.on_wait) if si is not None and si.on_wait else []
                if len(waits) > limit:
                    for w in waits[:-limit]:
                        nop = mybir.InstNoOp(
                            name=f"I-wsplit-{n_new}",
                            engine=inst.engine,
                            sync_info=mybir.SyncInfo(on_wait=[w], on_update=[]),
                        )
                        n_new += 1
                        out.append(nop)
                    si.on_wait = waits[-limit:]
                    changed = True
                out.append(inst)
            if changed:
                bb.instructions = out
    return n_new


def _build_program():
    key = "v17"
    if key in _PROG:
        return _PROG[key]
    import concourse.bass as bass
    import concourse.mybir as mybir

    F32 = mybir.dt.float32
    BF = mybir.dt.bfloat16
    n_warm = int(os.environ.get("BD_NWARM", "8"))

    nc = bass.Bass()
    qsT = nc.declare_dram_parameter("qsT", [128, W], BF, isOutput=False)
    ksT = nc.declare_dram_parameter("ksT", [128, W], BF, isOutput=False)
    k2n = nc.declare_dram_parameter("k2n", [128, W], BF, isOutput=False)
    hn = nc.declare_dram_parameter("hn", [128, W], BF, isOutput=False)
    tri = nc.declare_dram_parameter("tri", [128, 128], F32, isOutput=False)
    g128 = nc.declare_dram_parameter("g128", [128, 1], F32, isOutput=False)
    otT = nc.declare_dram_parameter("otT", [128, W], BF, isOutput=True)

    mm = nc.tensor.matmul
    with _patched_tc(nc) as tc:
        with tc.tile_pool(name="big", bufs=1) as big, \
             tc.tile_pool(name="small", bufs=1) as small, \
             tc.tile_pool(name="st", bufs=12) as stp, \
             tc.tile_pool(name="amp", bufs=3) as amp, \
             tc.tile_pool(name="ps_at", bufs=2, space="PSUM") as ps_at, \
             tc.tile_pool(name="ps_ot", bufs=2, space="PSUM") as ps_ot, \
             tc.tile_pool(name="ps_kp", bufs=2, space="PSUM") as ps_kp:

            qsT_sb = big.tile([128, W], BF, tag="qsT")
            ksT_sb = big.tile([128, W], BF, tag="ksT")
            k2n_sb = big.tile([128, W], BF, tag="k2n")
            hn_sb = big.tile([128, W], BF, tag="hn")
            otT_sb = big.tile([128, W], BF, tag="otT")
            tri_sb = small.tile([128, 128], F32, tag="tri")
            tri8_sb = small.tile([128, 1024], BF, tag="tri8")
            g128_sb = small.tile([128, 1], F32, tag="g128")

            # PE warm-up: dummy matmuls fill the DMA-wait window and flip
            # the HAM clock gate to 8/8 before the real stream starts.
            wz = small.tile([128, 512], BF, tag="wz")
            nc.vector.memset(wz[:], 0.0)
            for _ in range(n_warm):
                wp = ps_ot.tile([128, 512], F32, tag="ot")
                mm(wp[:], wz[:, :128], wz[:], start=True, stop=True)

            # inputs split across sync (k2n, ksT) and gpsimd (hn, qsT);
            # consts + output pieces on scalar's separate HWDGE queue
            nc.scalar.dma_start(tri_sb[:], tri[:])
            nc.scalar.dma_start(g128_sb[:], g128[:])
            for p in range(len(IN_BOUNDS) - 1):
                s = slice(IN_BOUNDS[p], IN_BOUNDS[p + 1])
                nc.sync.dma_start(k2n_sb[:, s], k2n[:, s])
                nc.gpsimd.dma_start(hn_sb[:, s], hn[:, s])
                nc.sync.dma_start(ksT_sb[:, s], ksT[:, s])
                nc.gpsimd.dma_start(qsT_sb[:, s], qsT[:, s])

            # tri8 = bf16 causal mask replicated x8 (doubling build)
            nc.scalar.copy(tri8_sb[:, 0:128], tri_sb[:])
            nc.scalar.copy(tri8_sb[:, 128:256], tri8_sb[:, 0:128])
            nc.scalar.copy(tri8_sb[:, 256:512], tri8_sb[:, 0:256])
            nc.scalar.copy(tri8_sb[:, 512:1024], tri8_sb[:, 0:512])

            S_prev = stp.tile([128, 128], BF, tag="S")
            nc.vector.memset(S_prev[:], 0.0)
            S_at = {0: S_prev}

            pend = None
            for (a, b) in G_MSK:
                n = b - a
                atb = ps_at.tile([128, 1024], F32, tag="at")

                # state path first: K' matmul + serial DVE update, per
                # chunk, so the chain advances as soon as data lands
                for m in range(a, b):
                    jj = slice(m * 128, (m + 1) * 128)
                    kp = ps_kp.tile([128, 128], F32, tag="kp")
                    mm(kp[:], k2n_sb[:, jj], hn_sb[:, jj], start=True,
                       stop=True)
                    S_new = stp.tile([128, 128], BF, tag="S")
                    nc.vector.scalar_tensor_tensor(
                        out=S_new[:], in0=S_at[m][:], scalar=g128_sb[:, 0:1],
                        in1=kp[:], op0=mybir.AluOpType.mult,
                        op1=mybir.AluOpType.add)
                    S_at[m + 1] = S_new

                # A blocks + one batched mask-multiply
                for m in range(a, b):
                    jj = slice(m * 128, (m + 1) * 128)
                    u = (m - a) * 128
                    mm(atb[:, u:u + 128], ksT_sb[:, jj], qsT_sb[:, jj],
                       start=True, stop=True)
                am = amp.tile([128, 1024], BF, tag="am")
                nc.vector.tensor_mul(am[:, :n * 128], atb[:, :n * 128],
                                     tri8_sb[:, :n * 128])

                # output blocks one group late: PE stays busy with the
                # next group's matmuls while the mask runs on DVE
                if pend is not None:
                    _emit_out(nc, mm, pend, hn_sb, qsT_sb, otT_sb, otT,
                              ps_ot, S_at)
                pend = (a, b, am)
            _emit_out(nc, mm, pend, hn_sb, qsT_sb, otT_sb, otT, ps_ot, S_at)

    _split_multi_waits(nc)
    _PROG[key] = nc
    return nc


def _emit_out(nc, mm, pend, hn_sb, qsT_sb, otT_sb, otT, ps_ot, S_at):
    import concourse.mybir as mybir
    a, b, am = pend
    for (ca, cb) in [g for g in G_OUT if a <= g[0] and g[1] <= b]:
        cn = cb - ca
        ot = ps_ot.tile([128, 512], mybir.dt.float32, tag="ot")
        for m in range(ca, cb):
            jj = slice(m * 128, (m + 1) * 128)
            osl = slice((m - ca) * 128, (m - ca + 1) * 128)
            asl = slice((m - a) * 128, (m - a + 1) * 128)
            mm(ot[:, osl], hn_sb[:, jj], am[:, asl], start=True, stop=False)
            mm(ot[:, osl], S_at[m][:], qsT_sb[:, jj], start=False, stop=True)
        oj = slice(ca * 128, cb * 128)
        nc.scalar.copy(otT_sb[:, oj], ot[:, :cn * 128])
        if cb in _OUT_PIECES:
            lo, hi = _OUT_PIECES[cb]
            s = slice(lo * 128, hi * 128)
            nc.scalar.dma_start(otT[:, s], otT_sb[:, s])


def _host_prep(q_alpha, k, h_norm, gamma_vec, causal_mask):
    import ml_dtypes
    bf = ml_dtypes.bfloat16
    gamma = np.clip(np.asarray(gamma_vec, np.float64), 1e-8, None)
    log_g = np.log(gamma)
    i_loc = (np.arange(W) % C).astype(np.float64)
    Sq = np.exp(np.outer(i_loc, log_g))          # [W, R] gamma^(i%C)
    Skneg = np.exp(np.outer(-i_loc, log_g))      # gamma^-(j%C)
    Sk2 = np.exp(np.outer(C - i_loc, log_g))     # gamma^(C - j%C)
    g128 = np.exp(C * log_g).astype(np.float32).reshape(128, 1)

    tri = np.ascontiguousarray(np.asarray(causal_mask, np.float32).T)

    def blockify(x):  # [W, 128] -> [128, (blk, 128)]
        return np.ascontiguousarray(
            x.reshape(NBLK, 128, 128).transpose(1, 0, 2).reshape(128, W))

    in_maps = []
    for b in range(B):
        q64 = np.asarray(q_alpha[b], np.float64)
        k64 = np.asarray(k[b], np.float64)
        in_maps.append({
            "qsT": np.ascontiguousarray((q64 * Sq).T.astype(bf)),
            "ksT": np.ascontiguousarray((k64 * Skneg).T.astype(bf)),
            "k2n": blockify((k64 * Sk2).astype(bf)),
            "hn": blockify(np.asarray(h_norm[b], bf)),
            "tri": tri,
            "g128": g128,
        })
    return in_maps


def _ensure_ntff_hook():
    try:
        from antenv import axon_hooks  # noqa: F401
        return
    except ImportError:
        pass
    import types
    import antenv
    try:
        import trn_agent_boot.trn_boot as tb
        hook = tb._ntff_profile_via_ctypes("/opt/axon/libaxon_pjrt.so")
    except Exception:
        hook = None
    mod = types.ModuleType("antenv.axon_hooks")
    mod.get_axon_ntff_profile_hook = lambda: hook
    mod.set_axon_ntff_profile_hook = lambda h: None
    sys.modules["antenv.axon_hooks"] = mod
    antenv.axon_hooks = mod


_last = {"exec_time_ns": None}


def kernel(q_alpha, k, h_norm, gamma_vec, causal_mask, decay_diff,
           _trace=False):
    trace = _trace or os.environ.get("BD_TRACE", "0") == "1"
    from concourse.bass_utils import run_bass_kernel_spmd

    nc = _build_program()
    in_maps = _host_prep(q_alpha, k, h_norm, gamma_vec, causal_mask)
    kwargs = {}
    if trace:
        _ensure_ntff_hook()
        import concourse.bass_utils as bu
        bu.upload_artifacts = lambda tmpdir: tmpdir  # no bucket in container
        kwargs = dict(trace=True, tmpdir=os.environ.get("BD_TRACE_DIR") or None)
    res = run_bass_kernel_spmd(nc, in_maps, list(range(B)), **kwargs)
    _last["exec_time_ns"] = res.exec_time_ns
    out = np.empty((B, W, D), np.float32)
    for b in range(B):
        out[b] = res.results[b]["otT"].T.astype(np.float32)
    return out
